# revision 5
# baseline (speedup 1.0000x reference)
"""Trainium2 Bass kernel for nn_CenterCrop: per-sample resize(short-side=256)
+ center-crop(224), bilinear, batch sharded over 8 NeuronCores.

Bilinear resize is separable: out = S^T @ img @ G with per-sample sparse
interpolation matrices S (vertical) and G (horizontal), built on the host
from the h/w metadata. The gather+lerp itself runs on the PE array as fp32
matmuls (exact):
  pass1: tmp1_T[x, j] = sum_y img[y, x] * S[y, j]   (img tiles stationary)
  pass2: out[j, i]    = sum_x tmp1_T[x, j] * G[x, i] (tmp1 tiles stationary)

Perf structure:
- Only the per-sample source window [y0min:y1max, x0min:x1max] that the
  output actually reads (~0.875*min(h,w) squared, 35-80% of the image) is
  DMA'd and processed.
- S/G are banded: each 128-row K-tile only touches a narrow output column
  range, so every matmul streams just that band (PSUM has_written bits make
  split accumulation exact).
- SPMD requires one program for all 8 cores, so samples are sorted by
  min(h,w) and dealt round-robin: slot s on every core holds same-sized
  windows; the program is specialized per-slot to the union shape/bands.
  Outputs are unpermuted on the host.
"""

import sys
import os

for _p in ("/opt/trn_rl_repo",):
    if os.path.isdir(_p) and _p not in sys.path:
        sys.path.insert(0, _p)

import numpy as np

OUT_H = 224
OUT_W = 224
RESIZE_TO = np.float32(256.0)
B_FULL = 64
N_CORES = 8
B_LOC = B_FULL // N_CORES  # 8 slots per core
C = 3
H = 512
W = 512  # image width after stripping the metadata column (stored width 513)

LAST_EXEC_NS = None
LAST_RESULTS = None
_NC_CACHE = {}


def _interp_matrices(h, w):
    """Full S [512, OUT_H], G [512, OUT_W] fp32 interpolation matrices,
    mirroring the reference fp32 math bit-for-bit."""
    f32 = np.float32
    h = f32(h)
    w = f32(w)
    min_dim = min(h, w)
    scale = RESIZE_TO / min_dim
    h_res = np.round(h * scale)
    w_res = np.round(w * scale)
    top = np.round((h_res - f32(OUT_H)) / f32(2.0))
    left = np.round((w_res - f32(OUT_W)) / f32(2.0))

    def axis_mat(n_out, offset, dim, dim_res, n_src):
        idx = np.arange(n_out, dtype=np.float32) + offset
        src = np.clip((idx + f32(0.5)) * dim / dim_res - f32(0.5),
                      f32(0.0), dim - f32(1.0))
        p0f = np.floor(src)
        frac = src - p0f
        imax = np.int32(dim) - 1
        p0 = np.clip(p0f.astype(np.int32), 0, imax)
        p1 = np.minimum(p0 + 1, imax)
        mat = np.zeros((n_src, n_out), np.float32)
        cols = np.arange(n_out)
        np.add.at(mat, (p0, cols), f32(1.0) - frac)
        np.add.at(mat, (p1, cols), frac)
        return mat

    S = axis_mat(OUT_H, top, h, h_res, H)
    G = axis_mat(OUT_W, left, w, w_res, W)
    return S, G


def _bands(mat_w, n_tiles):
    """Per-128-row-tile [lo, hi) columns with any nonzero; None if empty."""
    out = []
    for t in range(n_tiles):
        rows = mat_w[t * 128:(t + 1) * 128]
        nz = np.nonzero(rows.any(axis=0))[0]
        out.append(None if nz.size == 0 else (int(nz[0]), int(nz[-1]) + 1))
    return out


def _union_bands(band_lists):
    n = len(band_lists[0])
    out = []
    for t in range(n):
        los = [b[t][0] for b in band_lists if b[t] is not None]
        his = [b[t][1] for b in band_lists if b[t] is not None]
        out.append(None if not los else (min(los), max(his)))
    return out


def _prepare(x):
    """Host prep: per-sample windows/matrices, sorted slot assignment,
    per-core packed inputs, and the per-slot program parameters."""
    h_all = x[:, 0, 0, -1].astype(np.float32)
    w_all = x[:, 1, 0, -1].astype(np.float32)

    samples = []
    for b in range(B_FULL):
        S, G = _interp_matrices(h_all[b], w_all[b])
        ynz = np.nonzero(S.any(axis=1))[0]
        xnz = np.nonzero(G.any(axis=1))[0]
        y0, y1 = int(ynz[0]), int(ynz[-1]) + 1
        x0, x1 = int(xnz[0]), int(xnz[-1]) + 1
        samples.append(dict(S=S[y0:y1], G=G[x0:x1], y0=y0, x0=x0,
                            wh=y1 - y0, ww=x1 - x0))

    order = np.argsort(np.minimum(h_all, w_all), kind="stable")
    # slot s, core c -> sample order[s*N_CORES + c]
    assign = [[int(order[s * N_CORES + c]) for c in range(N_CORES)]
              for s in range(B_LOC)]

    slot_params = []
    slot_data = []  # per slot: list over cores of (sid, Sw_pad, Gw_pad)
    for s in range(B_LOC):
        sids = assign[s]
        wh = max(samples[i]["wh"] for i in sids)
        ww = max(samples[i]["ww"] for i in sids)
        n_yt = (wh + 127) // 128
        n_xt = (ww + 127) // 128
        sb_list, gb_list, data = [], [], []
        for i in sids:
            sp = samples[i]
            Sw = np.zeros((n_yt * 128, OUT_H), np.float32)
            Sw[:sp["wh"]] = sp["S"]
            Gw = np.zeros((n_xt * 128, OUT_W), np.float32)
            Gw[:sp["ww"]] = sp["G"]
            sb_list.append(_bands(Sw, n_yt))
            gb_list.append(_bands(Gw, n_xt))
            data.append((i, Sw, Gw))
        sbands = _union_bands(sb_list)
        gbands = _union_bands(gb_list)
        slot_params.append((n_yt, n_xt, ww,
                            tuple(sbands), tuple(gbands)))
        slot_data.append(data)

    # pack per-core input maps
    in_maps = [{} for _ in range(N_CORES)]
    for s in range(B_LOC):
        n_yt, n_xt, ww, _, _ = slot_params[s]
        for c in range(N_CORES):
            sid, Sw, Gw = slot_data[s][c]
            sp = samples[sid]
            xw = np.zeros((C, n_yt * 128, ww), np.float32)
            xw[:, :sp["wh"], :sp["ww"]] = x[
                sid, :, sp["y0"]:sp["y0"] + sp["wh"],
                sp["x0"]:sp["x0"] + sp["ww"]]
            in_maps[c][f"xw{s}"] = xw
            in_maps[c][f"s{s}"] = np.ascontiguousarray(
                Sw.reshape(n_yt, 128, OUT_H).transpose(1, 0, 2))
            in_maps[c][f"g{s}"] = np.ascontiguousarray(
                Gw.reshape(n_xt, 128, OUT_W).transpose(1, 0, 2))
    return tuple(slot_params), in_maps, assign


def _build_nc(slot_params):
    import concourse.bacc as bacc
    import concourse.mybir as mybir
    import concourse.tile as tile

    dt = mybir.dt.float32
    nc = bacc.Bacc(
        "TRN2",
        target_bir_lowering=False,
        debug=False,
        enable_asserts=False,
        num_devices=N_CORES,
    )
    xw_in, s_in, g_in = [], [], []
    for s, (n_yt, n_xt, ww, _, _) in enumerate(slot_params):
        xw_in.append(nc.dram_tensor(f"xw{s}", [C, n_yt * 128, ww], dt,
                                    kind="ExternalInput"))
        s_in.append(nc.dram_tensor(f"s{s}", [128, n_yt, OUT_H], dt,
                                   kind="ExternalInput"))
        g_in.append(nc.dram_tensor(f"g{s}", [128, n_xt, OUT_W], dt,
                                   kind="ExternalInput"))
    out = nc.dram_tensor("out", [B_LOC, C, OUT_H, OUT_W], dt,
                         kind="ExternalOutput")

    # biggest slot first: its DMAs prefetch earliest and the kernel tail
    # drains on the smallest slot instead of the largest
    slot_order = sorted(range(len(slot_params)),
                        key=lambda s: -slot_params[s][0] * slot_params[s][2])

    with tile.TileContext(nc) as tc:
        with (
            tc.tile_pool(name="img", bufs=4) as img_pool,
            tc.tile_pool(name="sg", bufs=3) as sg_pool,
            tc.tile_pool(name="tmp", bufs=2) as tmp_pool,
            tc.tile_pool(name="outp", bufs=2) as out_pool,
            tc.tile_pool(name="ps1", bufs=3, space="PSUM") as ps1_pool,
            tc.tile_pool(name="ps2", bufs=3, space="PSUM") as ps2_pool,
        ):
            for s in slot_order:
                n_yt, n_xt, ww, sbands, gbands = slot_params[s]
                s_sb = sg_pool.tile([128, n_yt, OUT_H], dt, tag="s")
                g_sb = sg_pool.tile([128, n_xt, OUT_W], dt, tag="g")
                nc.sync.dma_start(s_sb[:], s_in[s][:])
                nc.sync.dma_start(g_sb[:], g_in[s][:])
                out_sb = out_pool.tile([112, C, 2, OUT_W], dt)
                s_emit = [t for t in range(n_yt) if sbands[t] is not None]
                g_emit = [t for t in range(n_xt) if gbands[t] is not None]
                for c in range(C):
                    img_sb = img_pool.tile([128, n_yt, ww], dt)
                    src = xw_in[s][c].rearrange("(t p) x -> p t x", p=128)
                    nc.sync.dma_start(img_sb[:], src)
                    tmp_sb = tmp_pool.tile([128, n_xt, OUT_H], dt)
                    for xb in range(n_xt):
                        xlo = xb * 128
                        xn = min(128, ww - xlo)
                        ps1 = ps1_pool.tile([128, OUT_H], dt)
                        for i_t, t in enumerate(s_emit):
                            lo, hi = sbands[t]
                            nc.tensor.matmul(
                                ps1[:xn, lo:hi],
                                img_sb[:, t, xlo:xlo + xn],
                                s_sb[:, t, lo:hi],
                                start=(i_t == 0),
                                stop=(i_t == len(s_emit) - 1),
                                skip_group_check=True,
                            )
                        nc.vector.tensor_copy(tmp_sb[:xn, xb, :], ps1[:xn, :])
                    for jb in range(2):
                        ps2 = ps2_pool.tile([112, OUT_W], dt)
                        for i_t, xb in enumerate(g_emit):
                            lo, hi = gbands[xb]
                            xn = min(128, ww - xb * 128)
                            nc.tensor.matmul(
                                ps2[:, lo:hi],
                                tmp_sb[:xn, xb, jb * 112:(jb + 1) * 112],
                                g_sb[:xn, xb, lo:hi],
                                start=(i_t == 0),
                                stop=(i_t == len(g_emit) - 1),
                                skip_group_check=True,
                            )
                        nc.vector.tensor_copy(out_sb[:, c, jb, :], ps2[:])
                dst = out[s].rearrange("c (b p) i -> p c b i", p=112)
                nc.sync.dma_start(dst, out_sb[:])
    nc.compile()
    return nc


def kernel(x, _trace=False):
    global LAST_EXEC_NS, LAST_RESULTS
    from concourse.bass_utils import run_bass_kernel_spmd

    x = np.ascontiguousarray(np.asarray(x), dtype=np.float32)
    assert x.shape == (B_FULL, C, H, W + 1), x.shape

    slot_params, in_maps, assign = _prepare(x)
    if slot_params not in _NC_CACHE:
        _NC_CACHE[slot_params] = _build_nc(slot_params)
    nc = _NC_CACHE[slot_params]

    res = run_bass_kernel_spmd(nc, in_maps, list(range(N_CORES)), trace=_trace)
    LAST_EXEC_NS = res.exec_time_ns
    LAST_RESULTS = res

    out_full = np.empty((B_FULL, C, OUT_H, OUT_W), np.float32)
    for s in range(B_LOC):
        for c in range(N_CORES):
            out_full[assign[s][c]] = res.results[c]["out"][s]
    return out_full


# revision 10
# speedup vs baseline: 1.5052x; 1.5052x over previous
"""Trainium2 Bass kernel for nn_CenterCrop: per-sample resize(short-side=256)
+ center-crop(224), bilinear, batch sharded over 8 NeuronCores.

Bilinear resize is separable: out = S^T @ img @ G with per-sample sparse
interpolation matrices S (vertical) and G (horizontal), built on the host
from the h/w metadata. The gather+lerp itself runs on the PE array as fp32
matmuls (exact):
  pass1: tmp1_T[x, j] = sum_y img[y, x] * S[y, j]   (img tiles stationary)
  pass2: out[j, i]    = sum_x tmp1_T[x, j] * G[x, i] (tmp1 tiles stationary)

Perf structure:
- Only the per-sample source window [y0min:y1max, x0min:x1max] that the
  output actually reads (~0.875*min(h,w) squared, 35-80% of the image) is
  DMA'd and processed.
- S/G are banded: each 128-row K-tile only touches a narrow output column
  range, so every matmul streams just that band (PSUM has_written bits make
  split accumulation exact).
- SPMD requires one program for all 8 cores, so samples are sorted by
  min(h,w) and dealt round-robin: slot s on every core holds same-sized
  windows; the program is specialized per-slot to the union shape/bands.
  Outputs are unpermuted on the host.
"""

import sys
import os

for _p in ("/opt/trn_rl_repo",):
    if os.path.isdir(_p) and _p not in sys.path:
        sys.path.insert(0, _p)

import numpy as np

OUT_H = 224
OUT_W = 224
RESIZE_TO = np.float32(256.0)
B_FULL = 64
N_CORES = 8
B_LOC = B_FULL // N_CORES  # 8 slots per core
C = 3
H = 512
W = 512  # image width after stripping the metadata column (stored width 513)

LAST_EXEC_NS = None
LAST_RESULTS = None
_NC_CACHE = {}

# float32r experiment: single-pass PE fp32 (TF32-like rate) with streams
# padded to 256 cols to hit the fast path. Gated off by default; flip via
# env CENTERCROP_F32R=1 to measure precision/speed.
USE_F32R = os.environ.get("CENTERCROP_F32R", "0") == "1"
PADN = 256  # padded stream width under f32r


def _interp_matrices(h, w):
    """Full S [512, OUT_H], G [512, OUT_W] fp32 interpolation matrices,
    mirroring the reference fp32 math bit-for-bit."""
    f32 = np.float32
    h = f32(h)
    w = f32(w)
    min_dim = min(h, w)
    scale = RESIZE_TO / min_dim
    h_res = np.round(h * scale)
    w_res = np.round(w * scale)
    top = np.round((h_res - f32(OUT_H)) / f32(2.0))
    left = np.round((w_res - f32(OUT_W)) / f32(2.0))

    def axis_mat(n_out, offset, dim, dim_res, n_src):
        idx = np.arange(n_out, dtype=np.float32) + offset
        src = np.clip((idx + f32(0.5)) * dim / dim_res - f32(0.5),
                      f32(0.0), dim - f32(1.0))
        p0f = np.floor(src)
        frac = src - p0f
        imax = np.int32(dim) - 1
        p0 = np.clip(p0f.astype(np.int32), 0, imax)
        p1 = np.minimum(p0 + 1, imax)
        mat = np.zeros((n_src, n_out), np.float32)
        cols = np.arange(n_out)
        np.add.at(mat, (p0, cols), f32(1.0) - frac)
        np.add.at(mat, (p1, cols), frac)
        return mat

    S = axis_mat(OUT_H, top, h, h_res, H)
    G = axis_mat(OUT_W, left, w, w_res, W)
    return S, G


def _bands(mat_w, n_tiles):
    """Per-128-row-tile [lo, hi) columns with any nonzero; None if empty."""
    out = []
    for t in range(n_tiles):
        rows = mat_w[t * 128:(t + 1) * 128]
        nz = np.nonzero(rows.any(axis=0))[0]
        out.append(None if nz.size == 0 else (int(nz[0]), int(nz[-1]) + 1))
    return out


def _union_bands(band_lists):
    n = len(band_lists[0])
    out = []
    for t in range(n):
        los = [b[t][0] for b in band_lists if b[t] is not None]
        his = [b[t][1] for b in band_lists if b[t] is not None]
        out.append(None if not los else (min(los), max(his)))
    return out


def _prepare(x):
    """Host prep: per-sample windows/matrices, sorted slot assignment,
    per-core packed inputs, and the per-slot program parameters."""
    h_all = x[:, 0, 0, -1].astype(np.float32)
    w_all = x[:, 1, 0, -1].astype(np.float32)

    samples = []
    for b in range(B_FULL):
        S, G = _interp_matrices(h_all[b], w_all[b])
        ynz = np.nonzero(S.any(axis=1))[0]
        xnz = np.nonzero(G.any(axis=1))[0]
        y0, y1 = int(ynz[0]), int(ynz[-1]) + 1
        x0, x1 = int(xnz[0]), int(xnz[-1]) + 1
        samples.append(dict(S=S[y0:y1], G=G[x0:x1], y0=y0, x0=x0,
                            wh=y1 - y0, ww=x1 - x0))

    order = np.argsort(np.minimum(h_all, w_all), kind="stable")
    # slot s, core c -> sample order[s*N_CORES + c]
    assign = [[int(order[s * N_CORES + c]) for c in range(N_CORES)]
              for s in range(B_LOC)]

    slot_params = []
    slot_data = []  # per slot: list over cores of (sid, Sw_pad, Gw_pad)
    for s in range(B_LOC):
        sids = assign[s]
        wh = max(samples[i]["wh"] for i in sids)
        ww = max(samples[i]["ww"] for i in sids)
        n_yt = (wh + 127) // 128
        n_xt = (ww + 127) // 128
        sb_list, gb_list, data = [], [], []
        for i in sids:
            sp = samples[i]
            Sw = np.zeros((n_yt * 128, OUT_H), np.float32)
            Sw[:sp["wh"]] = sp["S"]
            Gw = np.zeros((n_xt * 128, OUT_W), np.float32)
            Gw[:sp["ww"]] = sp["G"]
            sb_list.append(_bands(Sw, n_yt))
            gb_list.append(_bands(Gw, n_xt))
            data.append((i, Sw, Gw))
        sbands = _union_bands(sb_list)
        gbands = _union_bands(gb_list)
        slot_params.append((n_yt, n_xt, ww,
                            tuple(sbands), tuple(gbands)))
        slot_data.append(data)

    # pack per-core input maps
    sgw = PADN if USE_F32R else OUT_H
    in_maps = [{} for _ in range(N_CORES)]
    for s in range(B_LOC):
        n_yt, n_xt, ww, _, _ = slot_params[s]
        for c in range(N_CORES):
            sid, Sw, Gw = slot_data[s][c]
            sp = samples[sid]
            xw = np.zeros((C, n_yt * 128, ww), np.float32)
            xw[:, :sp["wh"], :sp["ww"]] = x[
                sid, :, sp["y0"]:sp["y0"] + sp["wh"],
                sp["x0"]:sp["x0"] + sp["ww"]]
            in_maps[c][f"xw{s}"] = xw
            st = np.zeros((128, n_yt, sgw), np.float32)
            st[:, :, :OUT_H] = Sw.reshape(n_yt, 128, OUT_H).transpose(1, 0, 2)
            gt = np.zeros((128, n_xt, sgw), np.float32)
            gt[:, :, :OUT_W] = Gw.reshape(n_xt, 128, OUT_W).transpose(1, 0, 2)
            in_maps[c][f"s{s}"] = st
            in_maps[c][f"g{s}"] = gt
    return tuple(slot_params), in_maps, assign


def _build_nc(slot_params):
    import concourse.bacc as bacc
    import concourse.mybir as mybir
    import concourse.tile as tile

    dt = mybir.dt.float32
    dtd = mybir.dt.float32r if USE_F32R else mybir.dt.float32
    sgw = PADN if USE_F32R else OUT_H
    nc = bacc.Bacc(
        "TRN2",
        target_bir_lowering=False,
        debug=False,
        enable_asserts=False,
        num_devices=N_CORES,
    )
    xw_in, s_in, g_in = [], [], []
    for s, (n_yt, n_xt, ww, _, _) in enumerate(slot_params):
        xw_in.append(nc.dram_tensor(f"xw{s}", [C, n_yt * 128, ww], dtd,
                                    kind="ExternalInput"))
        s_in.append(nc.dram_tensor(f"s{s}", [128, n_yt, sgw], dtd,
                                   kind="ExternalInput"))
        g_in.append(nc.dram_tensor(f"g{s}", [128, n_xt, sgw], dtd,
                                   kind="ExternalInput"))
    out = nc.dram_tensor("out", [B_LOC, C, OUT_H, OUT_W], dt,
                         kind="ExternalOutput")

    # biggest slot first: its DMAs prefetch earliest and the kernel tail
    # drains on the smallest slot instead of the largest
    slot_order = sorted(range(len(slot_params)),
                        key=lambda s: -slot_params[s][0] * slot_params[s][2])

    with tile.TileContext(nc) as tc:
        with (
            tc.tile_pool(name="img", bufs=4) as img_pool,
            tc.tile_pool(name="sg", bufs=3) as sg_pool,
            tc.tile_pool(name="tmp", bufs=2) as tmp_pool,
            tc.tile_pool(name="outp", bufs=2) as out_pool,
            tc.tile_pool(name="ps1", bufs=3, space="PSUM") as ps1_pool,
            tc.tile_pool(name="ps2", bufs=3, space="PSUM") as ps2_pool,
        ):
            for s in slot_order:
                n_yt, n_xt, ww, sbands, gbands = slot_params[s]
                s_sb = sg_pool.tile([128, n_yt, sgw], dtd, tag="s")
                g_sb = sg_pool.tile([128, n_xt, sgw], dtd, tag="g")
                nc.sync.dma_start(s_sb[:], s_in[s][:])
                nc.sync.dma_start(g_sb[:], g_in[s][:])
                out_sb = out_pool.tile([112, C, 2, OUT_W], dt)
                s_emit = [t for t in range(n_yt) if sbands[t] is not None]
                g_emit = [t for t in range(n_xt) if gbands[t] is not None]
                for c in range(C):
                    img_sb = img_pool.tile([128, n_yt, ww], dtd)
                    src = xw_in[s][c].rearrange("(t p) x -> p t x", p=128)
                    nc.sync.dma_start(img_sb[:], src)
                    tmp_sb = tmp_pool.tile([128, n_xt, OUT_H], dtd)
                    for xb in range(n_xt):
                        xlo = xb * 128
                        xn = min(128, ww - xlo)
                        ps1 = ps1_pool.tile([128, sgw], dt)
                        for i_t, t in enumerate(s_emit):
                            lo, hi = (0, sgw) if USE_F32R else sbands[t]
                            nc.tensor.matmul(
                                ps1[:xn, lo:hi],
                                img_sb[:, t, xlo:xlo + xn],
                                s_sb[:, t, lo:hi],
                                start=(i_t == 0),
                                stop=(i_t == len(s_emit) - 1),
                                skip_group_check=True,
                            )
                        nc.vector.tensor_copy(tmp_sb[:xn, xb, :],
                                              ps1[:xn, :OUT_H])
                    for jb in range(2):
                        ps2 = ps2_pool.tile([112, sgw], dt)
                        for i_t, xb in enumerate(g_emit):
                            lo, hi = (0, sgw) if USE_F32R else gbands[xb]
                            xn = min(128, ww - xb * 128)
                            nc.tensor.matmul(
                                ps2[:, lo:hi],
                                tmp_sb[:xn, xb, jb * 112:(jb + 1) * 112],
                                g_sb[:xn, xb, lo:hi],
                                start=(i_t == 0),
                                stop=(i_t == len(g_emit) - 1),
                                skip_group_check=True,
                            )
                        nc.vector.tensor_copy(out_sb[:, c, jb, :],
                                              ps2[:, :OUT_W])
                dst = out[s].rearrange("c (b p) i -> p c b i", p=112)
                nc.sync.dma_start(dst, out_sb[:])
    nc.compile()
    return nc


def kernel(x, _trace=False):
    global LAST_EXEC_NS, LAST_RESULTS
    from concourse.bass_utils import run_bass_kernel_spmd

    x = np.ascontiguousarray(np.asarray(x), dtype=np.float32)
    assert x.shape == (B_FULL, C, H, W + 1), x.shape

    slot_params, in_maps, assign = _prepare(x)
    key = (slot_params, USE_F32R)
    if key not in _NC_CACHE:
        _NC_CACHE[key] = _build_nc(slot_params)
    nc = _NC_CACHE[key]

    res = run_bass_kernel_spmd(nc, in_maps, list(range(N_CORES)), trace=_trace)
    LAST_EXEC_NS = res.exec_time_ns
    LAST_RESULTS = res

    out_full = np.empty((B_FULL, C, OUT_H, OUT_W), np.float32)
    for s in range(B_LOC):
        for c in range(N_CORES):
            out_full[assign[s][c]] = res.results[c]["out"][s]
    return out_full
